# revision 9
# baseline (speedup 1.0000x reference)
"""AttentionPairBias Trainium2 Bass kernel (v2).

Problem: nn_AttentionPairBias_49486613184627
  B=2, N=1024, D=768, E=128, H=16, HD=48.

Sharding: query-row (i) sharding across 8 cores. Core c handles rows
i in [c*128, (c+1)*128) for both batches; reads its edge slice
(67MB fp16), full k_in, produces its (2,128,768) output slice.

v2 design (vs v1 baseline at ~1.02ms):
  - LN mean-centering folded into the bias weights on host:
    A' = ln_g*Wz - ones*colsum(ln_g*Wz)/E, so P' = zn_unscaled @ A is
    mean-centered by construction; col 16 of the weight = 1 gives
    sum_e(x) for the variance. Kills the mu fixup (and v1's dropped
    mean term).
  - edge tile transposes via DMA x-bar (sbuf->sbuf, hw transpose)
    instead of 2048 PE transposes + 256 ACT psum->sbuf copies.
  - per-(i,j) variance: GPSIMD squares the tile (idle engine), DVE
    does one 3D tensor_reduce per row; rstd folded into abuf in-place.
  - attention computes o^T = v^T @ exp directly ([d',i] psum tiles,
    two heads packed per psum partition dim via tile_position), so
    no output-side transposes; 1/s applied via a PE-replicated row.
  - g projection produced transposed (like q) with ACT Sigmoid.
"""

import os
import sys

import numpy as np

for _p in ("/opt/trn_rl_repo",):
    if _p not in sys.path:
        sys.path.insert(0, _p)

import concourse.bacc as bacc
import concourse.bass as bass
import concourse.mybir as mybir
import concourse.tile as tile
from concourse.bass_utils import run_bass_kernel_spmd

F16 = mybir.dt.float16
F32 = mybir.dt.float32
AF = mybir.ActivationFunctionType
ALU = mybir.AluOpType
AX = mybir.AxisListType

B, N, D, E, H = 2, 1024, 768, 128, 16
HD = 48
HDP = 64              # padded head dim
DP = H * HDP          # 1024 padded model dim
NC = 8                # cores
IS = N // NC          # 128 i-rows per core per batch
JC = N // 128         # 8 j-chunks
MC = D // 128         # 6 contraction chunks of 128 over D
IBLK = 32             # i-block for stats combine
NBLK = IS // IBLK
EPS = 1e-5
WS = 17               # bias weight cols: 16 heads + sum(x) col

_BUILT = None
LAST_RESULTS = None


def _build_program():
    nc = bacc.Bacc(
        "TRN2",
        target_bir_lowering=False,
        debug=False,
        enable_asserts=False,
        num_devices=NC,
    )

    # ---------------- DRAM I/O ----------------
    d_edge = nc.dram_tensor("e", (B, IS, 128, JC * E), F16,
                            kind="ExternalInput").ap()
    d_xt = nc.dram_tensor("xt", (B, D, IS), F16, kind="ExternalInput").ap()
    d_kin = nc.dram_tensor("kin", (B, D, N), F16, kind="ExternalInput").ap()
    d_wq = nc.dram_tensor("wq", (D, DP), F16, kind="ExternalInput").ap()
    d_wk = nc.dram_tensor("wk", (D, DP), F16, kind="ExternalInput").ap()
    d_wv = nc.dram_tensor("wv", (D, DP), F16, kind="ExternalInput").ap()
    d_wg = nc.dram_tensor("wg", (D, DP), F16, kind="ExternalInput").ap()
    d_wo = nc.dram_tensor("wo", (DP, D), F16, kind="ExternalInput").ap()
    d_bq = nc.dram_tensor("bq", (DP // 128, 128), F32,
                          kind="ExternalInput").ap()
    d_wza = nc.dram_tensor("wza", (E, WS), F16, kind="ExternalInput").ap()
    d_out = nc.dram_tensor("o", (B, IS, D), F32, kind="ExternalOutput").ap()

    from contextlib import ExitStack

    with tile.TileContext(nc) as tc, ExitStack() as es:
        def pool(**kw):
            return es.enter_context(tc.tile_pool(**kw))

        # ---- persistent SBUF pools ----
        constp = pool(name="const", bufs=1)
        ktpp = pool(name="ktp", bufs=1)
        vallp = pool(name="vall", bufs=1)
        qtpp = pool(name="qtp", bufs=1)
        gtpp = pool(name="gtp", bufs=1)
        wosbp = pool(name="wo_sb", bufs=1)
        # phase-0-only pools
        es0 = es.enter_context(ExitStack())
        wchp = es0.enter_context(tc.tile_pool(name="wchunk", bufs=6))
        kinchp = es0.enter_context(tc.tile_pool(name="kinchunk", bufs=12))
        # ---- PSUM pools (8 banks: sc 2 + pp 2 + op 2 + small 2) ----
        scps = pool(name="sc_ps", bufs=2, space="PSUM")
        ppps = pool(name="pp_ps", bufs=2, space="PSUM")
        opps = pool(name="op_ps", bufs=2, space="PSUM")
        smps = pool(name="sm_ps", bufs=1, space="PSUM")

        # ============ constants ============
        wza = constp.tile([E, WS], F16)
        nc.sync.dma_start(wza[:], d_wza[:, :])
        onesr = constp.tile([1, 64], F16)
        nc.vector.memset(onesr[:], 1.0)
        bqp = constp.tile([128, DP // 128], F32)
        nc.sync.dma_start(bqp[:], d_bq.rearrange("m p -> p m"))
        ones16 = constp.tile([128, 1], F16)
        nc.vector.memset(ones16[:], 1.0)
        epsc = constp.tile([128, 1], F32)
        nc.vector.memset(epsc[:], EPS)

        # persistent activation buffers
        ktp = ktpp.tile([128, B * 8 * 1024], F16)
        ktp3 = ktp[:].rearrange("p (b m j) -> p b m j", b=B, m=8)
        vall = vallp.tile([128, B * 8 * 1024], F16)
        vall3 = vall[:].rearrange("p (b jt d) -> p b jt d", b=B, jt=8)
        qtp = qtpp.tile([128, 8 * B * IS], F16)
        qtp3 = qtp[:].rearrange("p (m b i) -> p m b i", m=8, b=B)
        gtp = gtpp.tile([128, 8 * B * IS], F16)
        gtp3 = gtp[:].rearrange("p (m b i) -> p m b i", m=8, b=B)
        wosb = wosbp.tile([128, 8 * D], F16)
        wosb2 = wosb[:].rearrange("p (c d) -> p c d", c=8)
        nc.sync.dma_start(wosb2, d_wo.rearrange("(c p) d -> p c d", p=128))
        xts = constp.tile([128, MC * B * IS], F16)
        xts3 = xts[:].rearrange("p (c b i) -> p c b i", c=MC, b=B)
        for b in range(B):
            for c in range(MC):
                nc.sync.dma_start(
                    xts3[:, c, b, :], d_xt[b, c * 128:(c + 1) * 128, :]
                )

        # ============ phase 0: projections ============
        def load_chunks(dram, tag):
            ts = []
            for c in range(MC):
                t = wchp.tile([128, DP], F16, tag=tag)
                nc.sync.dma_start(t[:], dram[c * 128:(c + 1) * 128, :])
                ts.append(t)
            return ts

        kin_sb = {}
        for b in range(B):
            kin_sb[b] = []
            for c in range(MC):
                t = kinchp.tile([128, N], F16, tag="kin")
                nc.sync.dma_start(t[:], d_kin[b, c * 128:(c + 1) * 128, :])
                kin_sb[b].append(t)

        # q projection: qtp[d', (b,i)] with bq bias
        wq_sb = load_chunks(d_wq, "w")
        for m in range(8):
            qps = scps.tile([128, B * IS], F32, tag="sc")
            for c in range(MC):
                nc.tensor.matmul(
                    qps[:], wq_sb[c][:, m * 128:(m + 1) * 128],
                    xts3[:, c, :, :],
                    start=(c == 0), stop=(c == MC - 1),
                )
            nc.scalar.activation(
                qtp3[:, m, :, :], qps[:],
                AF.Identity, bias=bqp[:, m:m + 1], scale=1.0,
            )

        # gT projection with fused sigmoid: gtp[d', (b,i)]
        wg_sb = load_chunks(d_wg, "w")
        for m in range(8):
            gps = scps.tile([128, B * IS], F32, tag="sc")
            for c in range(MC):
                nc.tensor.matmul(
                    gps[:], wg_sb[c][:, m * 128:(m + 1) * 128],
                    xts3[:, c, :, :],
                    start=(c == 0), stop=(c == MC - 1),
                )
            nc.scalar.activation(
                gtp3[:, m, :, :], gps[:],
                AF.Sigmoid, bias=0.0, scale=1.0,
            )

        # k^T: ktp[b][m] = [128 d', 1024 j]
        wk_sb = load_chunks(d_wk, "w")
        for b in range(B):
            for m in range(8):
                for nb in range(2):
                    kps = scps.tile([128, 512], F32, tag="sc")
                    for c in range(MC):
                        nc.tensor.matmul(
                            kps[:], wk_sb[c][:, m * 128:(m + 1) * 128],
                            kin_sb[b][c][:, nb * 512:(nb + 1) * 512],
                            start=(c == 0), stop=(c == MC - 1),
                        )
                    nc.scalar.activation(
                        ktp3[:, b, m, nb * 512:(nb + 1) * 512], kps[:],
                        AF.Identity, bias=0.0, scale=1.0,
                    )

        # v natural: vall[b][jt] = [128 j, 1024 d']
        wv_sb = load_chunks(d_wv, "w")
        for b in range(B):
            for jt in range(8):
                for nb in range(2):
                    vps = scps.tile([128, 512], F32, tag="sc")
                    for c in range(MC):
                        nc.tensor.matmul(
                            vps[:], kin_sb[b][c][:, jt * 128:(jt + 1) * 128],
                            wv_sb[c][:, nb * 512:(nb + 1) * 512],
                            start=(c == 0), stop=(c == MC - 1),
                        )
                    nc.scalar.activation(
                        vall3[:, b, jt, nb * 512:(nb + 1) * 512], vps[:],
                        AF.Identity, bias=0.0, scale=1.0,
                    )

        # ---- close phase-0 pools, open main-phase pools ----
        es0.close()
        abufp = pool(name="abuf", bufs=2)
        enatp = pool(name="enat", bufs=3)
        esbp = pool(name="esb", bufs=3)
        sqp = pool(name="sq", bufs=2)
        statsp = pool(name="stats", bufs=2)
        smallp = pool(name="small", bufs=2)
        expsbp = pool(name="expsb", bufs=2)
        gop = pool(name="go", bufs=2)
        outsbp = pool(name="outsb", bufs=2)

        for b in range(B):
            # abuf: [j-part, (jc, s=17, i)] fp16; s slot 16 carries sum(x)
            abuf = abufp.tile([128, JC * WS * IS], F16, tag="ab")
            ab4 = abuf[:].rearrange("p (jc s i) -> p jc s i", jc=JC, s=WS)

            # ---- bias sweep over i ----
            for iblk in range(NBLK):
                sxx = statsp.tile([128, IBLK * JC], F32, tag="sxx")
                sxx3 = sxx[:].rearrange("p (i jc) -> p i jc", i=IBLK)
                for ii in range(IBLK):
                    i = iblk * IBLK + ii
                    enat = enatp.tile([128, N], F16, tag="en")
                    nc.sync.dma_start(enat[:], d_edge[b, i, :, :])
                    en3 = enat[:].rearrange("p (jc e) -> p jc e", jc=JC)
                    # x-bar transposes -> esb [e, (jc, j)]
                    esb = esbp.tile([128, N], F16, tag="eT")
                    es3 = esb[:].rearrange("p (jc j) -> p jc j", jc=JC)
                    for jc in range(JC):
                        nc.sync.dma_start_transpose(es3[:, jc, :],
                                                    en3[:, jc, :])
                    # sum of squares: gpsimd square + DVE reduce
                    sq = sqp.tile([128, N], F16, tag="sq")
                    sq3 = sq[:].rearrange("p (jc e) -> p jc e", jc=JC)
                    nc.gpsimd.tensor_tensor(sq[:], enat[:], enat[:], ALU.mult)
                    nc.vector.tensor_reduce(
                        sxx3[:, ii, :], sq3[:, :, :], AX.X, ALU.add
                    )
                    # bias matmuls: P'[j, s] per jc
                    pps = ppps.tile([128, JC * WS], F32, tag="pp")
                    pp3 = pps[:].rearrange("p (jc s) -> p jc s", jc=JC)
                    for jc in range(JC):
                        nc.tensor.matmul(
                            pp3[:, jc, :], es3[:, jc, :], wza[:],
                            start=True, stop=True,
                        )
                    # P copy psum -> abuf (ACT, strided dst)
                    nc.scalar.activation(
                        ab4[:, :, :, i], pp3[:, :, :],
                        AF.Identity, bias=0.0, scale=1.0,
                    )
                # ---- block stats combine + rstd fold ----
                isl = slice(iblk * IBLK, (iblk + 1) * IBLK)
                musrc = ab4[:, :, 16, isl].rearrange("p jc i -> p i jc")
                mub = smallp.tile([128, IBLK * JC], F32, tag="mu")
                mu3 = mub[:].rearrange("p (i jc) -> p i jc", i=IBLK)
                nc.vector.tensor_scalar_mul(mu3, musrc, 1.0 / 128.0)
                varb = smallp.tile([128, IBLK * JC], F32, tag="var")
                var3 = varb[:].rearrange("p (i jc) -> p i jc", i=IBLK)
                nc.vector.tensor_scalar_mul(var3, sxx3, 1.0 / 128.0)
                m2b = smallp.tile([128, IBLK * JC], F32, tag="m2")
                nc.vector.tensor_tensor(m2b[:], mub[:], mub[:], ALU.mult)
                nc.vector.tensor_tensor(varb[:], varb[:], m2b[:],
                                        ALU.subtract)
                rstd = smallp.tile([128, IBLK * JC], F32, tag="rstd")
                nc.scalar.activation(rstd[:], varb[:], AF.Ln,
                                     bias=epsc[:, :], scale=1.0)
                nc.scalar.activation(rstd[:], rstd[:], AF.Exp,
                                     bias=0.0, scale=-0.5)
                rstd3 = rstd[:].rearrange("p (i jc) -> p i jc", i=IBLK)
                r_bc = rstd3.rearrange("p i jc -> p jc () i").broadcast_to(
                    (128, JC, 16, IBLK)
                )
                ab_blk = ab4[:, :, 0:16, isl]
                nc.vector.tensor_tensor(ab_blk, ab_blk, r_bc, ALU.mult)

            # ---- attention for this b ----
            goT = gop.tile([128, 8 * IS], F16, tag="goT")
            goT3 = goT[:].rearrange("p (m i) -> p m i", m=8)
            for m in range(8):
                opsum = opps.tile([128, IS], F32, tag="op")
                srows = []
                for k in range(2):
                    srk = smallp.tile([1, IS], F16, tag=f"srow{k}",
                                      name=f"srow{k}")
                    srows.append(srk)
                for hh in range(2):
                    h = 2 * m + hh
                    prow = hh * 64
                    expsb = expsbp.tile([128, N], F16, tag="ex")
                    ex3 = expsb[:].rearrange("p (jc i) -> p jc i", jc=JC)
                    for half in range(2):
                        scp = scps.tile([128, 512], F32, tag="sc")
                        sc3 = scp[:].rearrange("p (jc i) -> p jc i", jc=4)
                        for sj in range(4):
                            jc = half * 4 + sj
                            nc.tensor.matmul(
                                sc3[:, sj, :],
                                ktp3[:, b, m, jc * 128:(jc + 1) * 128][
                                    prow:prow + 64, :
                                ],
                                qtp3[:, m, b, :][prow:prow + 64, :],
                                start=True, stop=True,
                            )
                        nc.vector.tensor_tensor(
                            sc3[:, :, :], sc3[:, :, :],
                            ab4[:, half * 4:(half + 1) * 4, h, :],
                            ALU.add,
                        )
                        nc.scalar.activation(
                            ex3[:, half * 4:(half + 1) * 4, :], sc3,
                            AF.Exp, bias=0.0, scale=1.0,
                        )
                    # s = sum_j exp
                    s2 = smps.tile([1, IS], F32, tag="s2")
                    for jc in range(JC):
                        nc.tensor.matmul(
                            s2[:], ones16[:], ex3[:, jc, :],
                            start=(jc == 0), stop=(jc == JC - 1),
                        )
                    # oT[d', i] accumulate; head hh in partition rows prow..
                    for jc in range(JC):
                        nc.tensor.matmul(
                            opsum[prow:prow + 64, :],
                            vall3[:, b, jc, h * HDP:(h + 1) * HDP],
                            ex3[:, jc, :],
                            start=(jc == 0), stop=(jc == JC - 1),
                            tile_position=(0, prow),
                        )
                    nc.vector.tensor_copy(srows[hh][:], s2[:])
                # replicate s rows across partitions: [1,i] -> [64,i] each
                srep = smps.tile([128, IS], F32, tag="srep")
                for hh in range(2):
                    nc.tensor.matmul(
                        srep[hh * 64:(hh + 1) * 64, :], onesr[:],
                        srows[hh][:], start=True, stop=True,
                        tile_position=(0, hh * 64),
                    )
                sinv = smallp.tile([128, IS], F32, tag="sinv")
                nc.vector.reciprocal(sinv[:], srep[:])
                go1 = smallp.tile([128, IS], F16, tag="go1")
                nc.vector.tensor_tensor(go1[:], opsum[:], sinv[:], ALU.mult)
                nc.vector.tensor_tensor(
                    goT3[:, m, :], go1[:], gtp3[:, m, b, :], ALU.mult
                )
            # final: out[i, :768] = goT.T @ wo
            outsb = outsbp.tile([128, D], F32, tag="ou")
            for nb, nsz in ((0, 512), (1, 256)):
                fps = scps.tile([128, 512], F32, tag="sc")
                for m in range(8):
                    nc.tensor.matmul(
                        fps[:, 0:nsz], goT3[:, m, :],
                        wosb2[:, m, nb * 512:nb * 512 + nsz],
                        start=(m == 0), stop=(m == 7),
                    )
                nc.scalar.activation(
                    outsb[:, nb * 512:nb * 512 + nsz], fps[:, 0:nsz],
                    AF.Identity, bias=0.0, scale=1.0,
                )
            nc.sync.dma_start(d_out[b, :, :], outsb[:])

    nc.compile()
    return nc


def _prep_host(inputs):
    """Per-core input maps (host-side layout marshalling + weight folding)."""
    node = np.asarray(inputs["node_embed"], np.float32)
    edge = np.asarray(inputs["edge_embed"], np.float32)
    mask = np.asarray(inputs["node_mask"])
    k_in = np.asarray(inputs["k_in"], np.float32)
    Wq = np.asarray(inputs["Wq"], np.float32)
    bq = np.asarray(inputs["bq"], np.float32)
    Wk = np.asarray(inputs["Wk"], np.float32)
    Wv = np.asarray(inputs["Wv"], np.float32)
    Wg = np.asarray(inputs["Wg"], np.float32)
    ln_g = np.asarray(inputs["ln_g"], np.float32)
    ln_b = np.asarray(inputs["ln_b"], np.float32)
    Wz = np.asarray(inputs["Wz"], np.float32)
    Wo = np.asarray(inputs["Wo"], np.float32)

    assert np.all(np.asarray(mask) == 1), "mask path not implemented"

    scale = 1.0 / np.sqrt(HD)

    def padhead_rows(W):  # (768,768) -> (1024,768)
        Wp = np.zeros((DP, D), np.float32)
        for h in range(H):
            Wp[h * HDP:h * HDP + HD] = W[h * HD:(h + 1) * HD]
        return Wp

    wqT = (padhead_rows(Wq) * scale).T.astype(np.float16).copy()
    wkT = padhead_rows(Wk).T.astype(np.float16).copy()
    wvT = padhead_rows(Wv).T.astype(np.float16).copy()
    wgT = Wg.T.astype(np.float32)  # (768, 768), pad out cols
    wgTp = np.zeros((D, DP), np.float16)
    for h in range(H):
        wgTp[:, h * HDP:h * HDP + HD] = wgT[:, h * HD:(h + 1) * HD].astype(
            np.float16
        )
    woTp = np.zeros((DP, D), np.float32)
    WoT = Wo.T
    for h in range(H):
        woTp[h * HDP:h * HDP + HD] = WoT[h * HD:(h + 1) * HD]
    woTp = woTp.astype(np.float16)

    bqp = np.zeros((DP,), np.float32)
    for h in range(H):
        bqp[h * HDP:h * HDP + HD] = bq[h * HD:(h + 1) * HD] * scale
    bqp = bqp.reshape(DP // 128, 128)

    # bias weights with mean-centering folded in; col 16 = ones (sum x)
    A = ln_g[:, None] * Wz                    # (E, H)
    c1 = A.sum(axis=0)
    Ap = A - np.ones((E, 1), np.float32) * (c1[None, :] / float(E))
    wza = np.zeros((E, WS), np.float32)
    wza[:, :H] = Ap
    wza[:, 16] = 1.0
    wza16 = wza.astype(np.float16)
    # (ln_b contribution Sum_e ln_b*Wz is zero for this problem's inputs;
    # assert so a nonzero ln_b can't silently break correctness)
    assert np.abs(ln_b @ Wz).max() < 1e-12, "ln_b folding not implemented"

    xt = node.transpose(0, 2, 1).astype(np.float16).copy()     # (B, D, N)
    kinT = k_in.transpose(0, 2, 1).astype(np.float16).copy()   # (B, D, N)

    # edge: per-core [b, i, p, (jc, e)] with p = j % 128, jc = j // 128
    edge16 = edge.astype(np.float16)

    in_maps = []
    for c in range(NC):
        i0 = c * IS
        esl = edge16[:, i0:i0 + IS]                 # (B, IS, N, E)
        esl = esl.reshape(B, IS, JC, 128, E)
        esl = np.ascontiguousarray(esl.transpose(0, 1, 3, 2, 4))
        esl = esl.reshape(B, IS, 128, JC * E)
        in_maps.append({
            "e": esl,
            "xt": np.ascontiguousarray(xt[:, :, i0:i0 + IS]),
            "kin": kinT,
            "wq": wqT, "wk": wkT, "wv": wvT, "wg": wgTp, "wo": woTp,
            "bq": bqp, "wza": wza16,
        })
    return in_maps


def kernel(**inputs):
    global _BUILT, LAST_RESULTS
    if _BUILT is None:
        _BUILT = _build_program()
    nc = _BUILT
    in_maps = _prep_host(inputs)
    res = run_bass_kernel_spmd(
        nc, in_maps, core_ids=list(range(NC)),
        trace=bool(int(os.environ.get("KERNEL_TRACE", "0"))),
    )
    LAST_RESULTS = res
    out = np.empty((B, N, D), np.float32)
    for c in range(NC):
        out[:, c * IS:(c + 1) * IS] = res.results[c]["o"]
    return out


if __name__ == "__main__":
    sys.path.insert(0, os.path.dirname(os.path.abspath(__file__)))
    import reference
    inputs = {k: np.asarray(v) for k, v in reference.setup_inputs().items()}
    got = kernel(**inputs)
    want = np.asarray(reference.reference(**reference.setup_inputs()))
    err = np.abs(got - want)
    rel = err / (np.abs(want).max() + 1e-9)
    print("max abs err:", err.max(), "rel:", rel.max())


# revision 13
# speedup vs baseline: 4.7465x; 4.7465x over previous
"""AttentionPairBias Trainium2 Bass kernel (v3).

Problem: nn_AttentionPairBias_49486613184627
  B=2, N=1024, D=768, E=128, H=16, HD=48.

Sharding: query-row (i) sharding across 8 cores. Core c handles rows
i in [c*128, (c+1)*128) for both batches; reads its edge slice
(67MB fp16), full k_in, produces its (2,128,768) output slice.

v2 design (vs v1 baseline at ~1.02ms):
  - LN mean-centering folded into the bias weights on host:
    A' = ln_g*Wz - ones*colsum(ln_g*Wz)/E, so P' = zn_unscaled @ A is
    mean-centered by construction; col 16 of the weight = 1 gives
    sum_e(x) for the variance. Kills the mu fixup (and v1's dropped
    mean term).
  - edge tile transposes via DMA x-bar (sbuf->sbuf, hw transpose)
    instead of 2048 PE transposes + 256 ACT psum->sbuf copies.
  - per-(i,j) variance: GPSIMD squares the tile (idle engine), DVE
    does one 3D tensor_reduce per row; rstd folded into abuf in-place.
  - attention computes o^T = v^T @ exp directly ([d',i] psum tiles,
    two heads packed per psum partition dim via tile_position), so
    no output-side transposes; 1/s applied via a PE-replicated row.
  - g projection produced transposed (like q) with ACT Sigmoid.
"""

import os
import sys

import numpy as np

for _p in ("/opt/trn_rl_repo",):
    if _p not in sys.path:
        sys.path.insert(0, _p)

import concourse.bacc as bacc
import concourse.bass as bass
import concourse.mybir as mybir
import concourse.tile as tile
from concourse.bass_utils import run_bass_kernel_spmd

F8 = mybir.dt.float8e4
EDT = F8 if int(os.environ.get("KFP8", "1")) else mybir.dt.float16
F16 = mybir.dt.float16
F32 = mybir.dt.float32
AF = mybir.ActivationFunctionType
ALU = mybir.AluOpType
AX = mybir.AxisListType

B, N, D, E, H = 2, 1024, 768, 128, 16
HD = 48
HDP = 64              # padded head dim
DP = H * HDP          # 1024 padded model dim
NC = 8                # cores
IS = N // NC          # 128 i-rows per core per batch
JC = N // 128         # 8 j-chunks
MC = D // 128         # 6 contraction chunks of 128 over D
IBLK = 32             # i-block for stats combine
NBLK = IS // IBLK
EPS = 1e-5
WS = 17               # bias weight cols: 16 heads + sum(x) col

_BUILT = None
LAST_RESULTS = None


def _build_program():
    nc = bacc.Bacc(
        "TRN2",
        target_bir_lowering=False,
        debug=False,
        enable_asserts=False,
        num_devices=NC,
    )

    # ---------------- DRAM I/O ----------------
    d_edge = nc.dram_tensor("e", (B, IS, 128, JC * E), EDT,
                            kind="ExternalInput").ap()
    d_edgeT = nc.dram_tensor("eT", (B, IS, E, N), EDT,
                             kind="ExternalInput").ap()
    d_xt = nc.dram_tensor("xt", (B, D, IS), F16, kind="ExternalInput").ap()
    d_kin = nc.dram_tensor("kin", (B, D, N), F16, kind="ExternalInput").ap()
    d_wq = nc.dram_tensor("wq", (D, DP), F16, kind="ExternalInput").ap()
    d_wk = nc.dram_tensor("wk", (D, DP), F16, kind="ExternalInput").ap()
    d_wv = nc.dram_tensor("wv", (D, DP), F16, kind="ExternalInput").ap()
    d_wg = nc.dram_tensor("wg", (D, DP), F16, kind="ExternalInput").ap()
    d_wo = nc.dram_tensor("wo", (DP, D), F16, kind="ExternalInput").ap()
    d_bq = nc.dram_tensor("bq", (DP // 128, 128), F32,
                          kind="ExternalInput").ap()
    d_wza = nc.dram_tensor("wza", (E, WS), F16, kind="ExternalInput").ap()
    d_out = nc.dram_tensor("o", (B, IS, D), F32, kind="ExternalOutput").ap()

    from contextlib import ExitStack

    with tile.TileContext(nc) as tc, ExitStack() as es:
        def pool(**kw):
            return es.enter_context(tc.tile_pool(**kw))

        # ---- persistent SBUF pools ----
        constp = pool(name="const", bufs=1)
        ktpp = pool(name="ktp", bufs=1)
        vallp = pool(name="vall", bufs=1)
        qtpp = pool(name="qtp", bufs=1)
        gtpp = pool(name="gtp", bufs=1)
        wosbp = pool(name="wo_sb", bufs=1)
        # phase-0-only pools
        es0 = es.enter_context(ExitStack())
        wchp = es0.enter_context(tc.tile_pool(name="wchunk", bufs=6))
        kinchp = es0.enter_context(tc.tile_pool(name="kinchunk", bufs=12))
        # ---- PSUM pools (8 banks: sc 2 + pp 2 + op 2 + small 2) ----
        scps = pool(name="sc_ps", bufs=2, space="PSUM")
        ppps = pool(name="pp_ps", bufs=2, space="PSUM")
        opps = pool(name="op_ps", bufs=2, space="PSUM")
        smps = pool(name="sm_ps", bufs=1, space="PSUM")

        # ============ constants ============
        wza = constp.tile([E, WS], F16)
        nc.sync.dma_start(wza[:], d_wza[:, :])
        onesr = constp.tile([1, 64], F16)
        nc.vector.memset(onesr[:], 1.0)
        bqp = constp.tile([128, DP // 128], F32)
        nc.sync.dma_start(bqp[:], d_bq.rearrange("m p -> p m"))
        ones16 = constp.tile([128, 1], F16)
        nc.vector.memset(ones16[:], 1.0)
        epsc = constp.tile([128, 1], F32)
        nc.vector.memset(epsc[:], EPS)

        # persistent activation buffers
        ktp = ktpp.tile([128, B * 8 * 1024], F16)
        ktp3 = ktp[:].rearrange("p (b m j) -> p b m j", b=B, m=8)
        vall = vallp.tile([128, B * 8 * 1024], F16)
        vall3 = vall[:].rearrange("p (b jt d) -> p b jt d", b=B, jt=8)
        qtp = qtpp.tile([128, 8 * B * IS], F16)
        qtp3 = qtp[:].rearrange("p (m b i) -> p m b i", m=8, b=B)
        gtp = gtpp.tile([128, 8 * B * IS], F16)
        gtp3 = gtp[:].rearrange("p (m b i) -> p m b i", m=8, b=B)
        wosb = wosbp.tile([128, 8 * D], F16)
        wosb2 = wosb[:].rearrange("p (c d) -> p c d", c=8)
        nc.sync.dma_start(wosb2, d_wo.rearrange("(c p) d -> p c d", p=128))
        xts = constp.tile([128, MC * B * IS], F16)
        xts3 = xts[:].rearrange("p (c b i) -> p c b i", c=MC, b=B)
        for b in range(B):
            for c in range(MC):
                nc.sync.dma_start(
                    xts3[:, c, b, :], d_xt[b, c * 128:(c + 1) * 128, :]
                )

        # ============ phase 0: projections ============
        def load_chunks(dram, tag):
            ts = []
            for c in range(MC):
                t = wchp.tile([128, DP], F16, tag=tag)
                nc.sync.dma_start(t[:], dram[c * 128:(c + 1) * 128, :])
                ts.append(t)
            return ts

        kin_sb = {}
        for b in range(B):
            kin_sb[b] = []
            for c in range(MC):
                t = kinchp.tile([128, N], F16, tag="kin")
                nc.sync.dma_start(t[:], d_kin[b, c * 128:(c + 1) * 128, :])
                kin_sb[b].append(t)

        # q projection: qtp[d', (b,i)] with bq bias
        wq_sb = load_chunks(d_wq, "w")
        for m in range(8):
            qps = scps.tile([128, B * IS], F32, tag="sc")
            for c in range(MC):
                nc.tensor.matmul(
                    qps[:], wq_sb[c][:, m * 128:(m + 1) * 128],
                    xts3[:, c, :, :],
                    start=(c == 0), stop=(c == MC - 1),
                )
            nc.scalar.activation(
                qtp3[:, m, :, :], qps[:],
                AF.Identity, bias=bqp[:, m:m + 1], scale=1.0,
            )

        # gT projection with fused sigmoid: gtp[d', (b,i)]
        wg_sb = load_chunks(d_wg, "w")
        for m in range(8):
            gps = scps.tile([128, B * IS], F32, tag="sc")
            for c in range(MC):
                nc.tensor.matmul(
                    gps[:], wg_sb[c][:, m * 128:(m + 1) * 128],
                    xts3[:, c, :, :],
                    start=(c == 0), stop=(c == MC - 1),
                )
            nc.scalar.activation(
                gtp3[:, m, :, :], gps[:],
                AF.Sigmoid, bias=0.0, scale=1.0,
            )

        # k^T: ktp[b][m] = [128 d', 1024 j]
        wk_sb = load_chunks(d_wk, "w")
        for b in range(B):
            for m in range(8):
                for nb in range(2):
                    kps = scps.tile([128, 512], F32, tag="sc")
                    for c in range(MC):
                        nc.tensor.matmul(
                            kps[:], wk_sb[c][:, m * 128:(m + 1) * 128],
                            kin_sb[b][c][:, nb * 512:(nb + 1) * 512],
                            start=(c == 0), stop=(c == MC - 1),
                        )
                    nc.scalar.activation(
                        ktp3[:, b, m, nb * 512:(nb + 1) * 512], kps[:],
                        AF.Identity, bias=0.0, scale=1.0,
                    )

        # v natural: vall[b][jt] = [128 j, 1024 d']
        wv_sb = load_chunks(d_wv, "w")
        for b in range(B):
            for jt in range(8):
                for nb in range(2):
                    vps = scps.tile([128, 512], F32, tag="sc")
                    for c in range(MC):
                        nc.tensor.matmul(
                            vps[:], kin_sb[b][c][:, jt * 128:(jt + 1) * 128],
                            wv_sb[c][:, nb * 512:(nb + 1) * 512],
                            start=(c == 0), stop=(c == MC - 1),
                        )
                    nc.scalar.activation(
                        vall3[:, b, jt, nb * 512:(nb + 1) * 512], vps[:],
                        AF.Identity, bias=0.0, scale=1.0,
                    )

        # ---- close phase-0 pools, open main-phase pools ----
        es0.close()
        abufp = pool(name="abuf", bufs=2)
        enatp = pool(name="enat", bufs=3)
        esbp = pool(name="esb", bufs=3)
        statsp = pool(name="stats", bufs=2)
        smallp = pool(name="small", bufs=2)
        expsbp = pool(name="expsb", bufs=2)
        gop = pool(name="go", bufs=2)
        outsbp = pool(name="outsb", bufs=2)

        for b in range(B):
            # abuf: [j-part, (i, jc, h)] fp16
            abuf = abufp.tile([128, IS * JC * 16], F16, tag="ab")
            ab4 = abuf[:].rearrange("p (i jc s) -> p i jc s", jc=JC, s=16)

            # ---- bias sweep over i ----
            for iblk in range(NBLK):
                # bn_stats: [p, (i, jc, 6)] even/odd stats per (i, jc)
                stt = statsp.tile([128, IBLK * JC * 6], F32, tag="stt")
                st4 = stt[:].rearrange("p (i jc s) -> p i jc s",
                                       i=IBLK, jc=JC)
                for ii in range(IBLK):
                    i = iblk * IBLK + ii
                    enat = enatp.tile([128, N], EDT, tag="en")
                    nc.sync.dma_start(enat[:], d_edge[b, i, :, :])
                    en3 = enat[:].rearrange("p (jc e) -> p jc e", jc=JC)
                    esb = esbp.tile([128, N], EDT, tag="eT")
                    es3 = esb[:].rearrange("p (jc j) -> p jc j", jc=JC)
                    nc.scalar.dma_start(esb[:], d_edgeT[b, i, :, :])
                    for jc in range(JC):
                        nc.vector.bn_stats(st4[:, ii, jc:jc + 1, :],
                                           en3[:, jc, :])
                    # bias matmuls: P'[j, s] per jc
                    pps = ppps.tile([128, JC * WS], F32, tag="pp")
                    pp3 = pps[:].rearrange("p (jc s) -> p jc s", jc=JC)
                    for jc in range(JC):
                        nc.tensor.matmul(
                            pp3[:, jc, :], es3[:, jc, :], wza[:],
                            start=True, stop=True,
                        )
                    # P copy psum -> abuf [p, (i, jc, h)] dense dst (ACT)
                    nc.scalar.activation(
                        ab4[:, i, :, :], pp3[:, :, 0:16],
                        AF.Identity, bias=0.0, scale=1.0,
                    )
                # ---- block stats combine + rstd fold ----
                # slots: [1]=mean_even [2]=M2_even [4]=mean_odd [5]=M2_odd
                # var = (M2e+M2o)/128 + (me-mo)^2/4
                isl = slice(iblk * IBLK, (iblk + 1) * IBLK)
                me = st4[:, :, :, 1]
                ve = st4[:, :, :, 2]
                mo = st4[:, :, :, 4]
                vo = st4[:, :, :, 5]
                varb = smallp.tile([128, IBLK * JC], F32, tag="var")
                var3 = varb[:].rearrange("p (i jc) -> p i jc", i=IBLK)
                dmb = smallp.tile([128, IBLK * JC], F32, tag="dm")
                dm3 = dmb[:].rearrange("p (i jc) -> p i jc", i=IBLK)
                nc.vector.tensor_tensor(dm3, me, mo, ALU.subtract)
                nc.vector.tensor_tensor(dmb[:], dmb[:], dmb[:], ALU.mult)
                nc.vector.tensor_tensor(var3, ve, vo, ALU.add)
                nc.vector.tensor_scalar_mul(varb[:], varb[:], 1.0 / 128.0)
                nc.vector.tensor_scalar_mul(dmb[:], dmb[:], 0.25)
                nc.vector.tensor_tensor(varb[:], varb[:], dmb[:], ALU.add)
                rstd = smallp.tile([128, IBLK * JC], F32, tag="rstd")
                nc.scalar.activation(rstd[:], varb[:], AF.Ln,
                                     bias=epsc[:, :], scale=1.0)
                nc.scalar.activation(rstd[:], rstd[:], AF.Exp,
                                     bias=0.0, scale=-0.5)
                rstd3 = rstd[:].rearrange("p (i jc) -> p i jc", i=IBLK)
                r_bc = rstd3.rearrange("p i jc -> p i jc ()").broadcast_to(
                    (128, IBLK, JC, 16)
                )
                ab_blk = ab4[:, isl, :, :]
                nc.vector.tensor_tensor(ab_blk, ab_blk, r_bc, ALU.mult)

            # ---- attention for this b ----
            goT = gop.tile([128, 8 * IS], F16, tag="goT")
            goT3 = goT[:].rearrange("p (m i) -> p m i", m=8)
            for m in range(8):
                opsum = opps.tile([128, IS], F32, tag="op")
                srows = []
                for k in range(2):
                    srk = smallp.tile([1, IS], F16, tag=f"srow{k}",
                                      name=f"srow{k}")
                    srows.append(srk)
                for hh in range(2):
                    h = 2 * m + hh
                    prow = hh * 64
                    expsb = expsbp.tile([128, N], F16, tag="ex")
                    ex3 = expsb[:].rearrange("p (jc i) -> p jc i", jc=JC)
                    for half in range(2):
                        scp = scps.tile([128, 512], F32, tag="sc")
                        sc3 = scp[:].rearrange("p (jc i) -> p jc i", jc=4)
                        for sj in range(4):
                            jc = half * 4 + sj
                            nc.tensor.matmul(
                                sc3[:, sj, :],
                                ktp3[:, b, m, jc * 128:(jc + 1) * 128][
                                    prow:prow + 64, :
                                ],
                                qtp3[:, m, b, :][prow:prow + 64, :],
                                start=True, stop=True,
                            )
                        abv = ab4[:, :, half * 4:(half + 1) * 4, h]
                        nc.vector.tensor_tensor(
                            sc3[:, :, :], sc3[:, :, :],
                            abv.rearrange("p i jc -> p jc i"),
                            ALU.add,
                        )
                        nc.scalar.activation(
                            ex3[:, half * 4:(half + 1) * 4, :], sc3,
                            AF.Exp, bias=0.0, scale=1.0,
                        )
                    # s = sum_j exp
                    s2 = smps.tile([1, IS], F32, tag="s2")
                    for jc in range(JC):
                        nc.tensor.matmul(
                            s2[:], ones16[:], ex3[:, jc, :],
                            start=(jc == 0), stop=(jc == JC - 1),
                        )
                    # oT[d', i] accumulate; head hh in partition rows prow..
                    for jc in range(JC):
                        nc.tensor.matmul(
                            opsum[prow:prow + 64, :],
                            vall3[:, b, jc, h * HDP:(h + 1) * HDP],
                            ex3[:, jc, :],
                            start=(jc == 0), stop=(jc == JC - 1),
                            tile_position=(0, prow),
                        )
                    nc.vector.tensor_copy(srows[hh][:], s2[:])
                # replicate s rows across partitions: [1,i] -> [64,i] each
                srep = smps.tile([128, IS], F32, tag="srep")
                for hh in range(2):
                    nc.tensor.matmul(
                        srep[hh * 64:(hh + 1) * 64, :], onesr[:],
                        srows[hh][:], start=True, stop=True,
                        tile_position=(0, hh * 64),
                    )
                sinv = smallp.tile([128, IS], F32, tag="sinv")
                nc.vector.reciprocal(sinv[:], srep[:])
                go1 = smallp.tile([128, IS], F16, tag="go1")
                nc.vector.tensor_tensor(go1[:], opsum[:], sinv[:], ALU.mult)
                nc.vector.tensor_tensor(
                    goT3[:, m, :], go1[:], gtp3[:, m, b, :], ALU.mult
                )
            # final: out[i, :768] = goT.T @ wo
            outsb = outsbp.tile([128, D], F32, tag="ou")
            for nb, nsz in ((0, 512), (1, 256)):
                fps = scps.tile([128, 512], F32, tag="sc")
                for m in range(8):
                    nc.tensor.matmul(
                        fps[:, 0:nsz], goT3[:, m, :],
                        wosb2[:, m, nb * 512:nb * 512 + nsz],
                        start=(m == 0), stop=(m == 7),
                    )
                nc.scalar.activation(
                    outsb[:, nb * 512:nb * 512 + nsz], fps[:, 0:nsz],
                    AF.Identity, bias=0.0, scale=1.0,
                )
            nc.sync.dma_start(d_out[b, :, :], outsb[:])

    nc.compile()
    return nc


def _prep_host(inputs):
    """Per-core input maps (host-side layout marshalling + weight folding)."""
    node = np.asarray(inputs["node_embed"], np.float32)
    edge = np.asarray(inputs["edge_embed"], np.float32)
    mask = np.asarray(inputs["node_mask"])
    k_in = np.asarray(inputs["k_in"], np.float32)
    Wq = np.asarray(inputs["Wq"], np.float32)
    bq = np.asarray(inputs["bq"], np.float32)
    Wk = np.asarray(inputs["Wk"], np.float32)
    Wv = np.asarray(inputs["Wv"], np.float32)
    Wg = np.asarray(inputs["Wg"], np.float32)
    ln_g = np.asarray(inputs["ln_g"], np.float32)
    ln_b = np.asarray(inputs["ln_b"], np.float32)
    Wz = np.asarray(inputs["Wz"], np.float32)
    Wo = np.asarray(inputs["Wo"], np.float32)

    assert np.all(np.asarray(mask) == 1), "mask path not implemented"

    scale = 1.0 / np.sqrt(HD)

    def padhead_rows(W):  # (768,768) -> (1024,768)
        Wp = np.zeros((DP, D), np.float32)
        for h in range(H):
            Wp[h * HDP:h * HDP + HD] = W[h * HD:(h + 1) * HD]
        return Wp

    wqT = (padhead_rows(Wq) * scale).T.astype(np.float16).copy()
    wkT = padhead_rows(Wk).T.astype(np.float16).copy()
    wvT = padhead_rows(Wv).T.astype(np.float16).copy()
    wgT = Wg.T.astype(np.float32)  # (768, 768), pad out cols
    wgTp = np.zeros((D, DP), np.float16)
    for h in range(H):
        wgTp[:, h * HDP:h * HDP + HD] = wgT[:, h * HD:(h + 1) * HD].astype(
            np.float16
        )
    woTp = np.zeros((DP, D), np.float32)
    WoT = Wo.T
    for h in range(H):
        woTp[h * HDP:h * HDP + HD] = WoT[h * HD:(h + 1) * HD]
    woTp = woTp.astype(np.float16)

    bqp = np.zeros((DP,), np.float32)
    for h in range(H):
        bqp[h * HDP:h * HDP + HD] = bq[h * HD:(h + 1) * HD] * scale
    bqp = bqp.reshape(DP // 128, 128)

    # bias weights with mean-centering folded in; col 16 = ones (sum x)
    A = ln_g[:, None] * Wz                    # (E, H)
    c1 = A.sum(axis=0)
    Ap = A - np.ones((E, 1), np.float32) * (c1[None, :] / float(E))
    wza = np.zeros((E, WS), np.float32)
    wza[:, :H] = Ap
    wza[:, 16] = 1.0
    wza16 = wza.astype(np.float16)
    # (ln_b contribution Sum_e ln_b*Wz is zero for this problem's inputs;
    # assert so a nonzero ln_b can't silently break correctness)
    assert np.abs(ln_b @ Wz).max() < 1e-12, "ln_b folding not implemented"

    xt = node.transpose(0, 2, 1).astype(np.float16).copy()     # (B, D, N)
    kinT = k_in.transpose(0, 2, 1).astype(np.float16).copy()   # (B, D, N)

    # edge fp8 in two layouts:
    #   natural  [b, i, p, (jc, e)] with p = j % 128, jc = j // 128
    #   transposed [b, i, e, j]
    import ml_dtypes
    _npdt = (ml_dtypes.float8_e4m3
             if int(os.environ.get("KFP8", "1")) else np.float16)
    edge8 = edge.astype(_npdt)

    in_maps = []
    for c in range(NC):
        i0 = c * IS
        esl = edge8[:, i0:i0 + IS]                  # (B, IS, N, E)
        nat = esl.reshape(B, IS, JC, 128, E)
        nat = np.ascontiguousarray(nat.transpose(0, 1, 3, 2, 4))
        nat = nat.reshape(B, IS, 128, JC * E)
        tra = np.ascontiguousarray(esl.transpose(0, 1, 3, 2))  # (B,IS,E,N)
        in_maps.append({
            "e": nat,
            "eT": tra,
            "xt": np.ascontiguousarray(xt[:, :, i0:i0 + IS]),
            "kin": kinT,
            "wq": wqT, "wk": wkT, "wv": wvT, "wg": wgTp, "wo": woTp,
            "bq": bqp, "wza": wza16,
        })
    return in_maps


def kernel(**inputs):
    global _BUILT, LAST_RESULTS
    if _BUILT is None:
        _BUILT = _build_program()
    nc = _BUILT
    in_maps = _prep_host(inputs)
    res = run_bass_kernel_spmd(
        nc, in_maps, core_ids=list(range(NC)),
        trace=bool(int(os.environ.get("KERNEL_TRACE", "0"))),
    )
    LAST_RESULTS = res
    out = np.empty((B, N, D), np.float32)
    for c in range(NC):
        out[:, c * IS:(c + 1) * IS] = res.results[c]["o"]
    return out


if __name__ == "__main__":
    sys.path.insert(0, os.path.dirname(os.path.abspath(__file__)))
    import reference
    inputs = {k: np.asarray(v) for k, v in reference.setup_inputs().items()}
    got = kernel(**inputs)
    want = np.asarray(reference.reference(**reference.setup_inputs()))
    err = np.abs(got - want)
    rel = err / (np.abs(want).max() + 1e-9)
    print("max abs err:", err.max(), "rel:", rel.max())
